# revision 7
# baseline (speedup 1.0000x reference)
"""Trainium2 kernel for nn_BCE_35270271435550.

Pipeline: conv3d(32->1) -> fuzzy Gaussian membership -> BN1 -> conv3d(1->32)
-> BN2, fully on 8 NeuronCores (z-sharded), with two tiny DRAM AllReduces for
the BatchNorm statistics.

Math folds (host, exact):
- fuzzy membership exp(-sum_j ((x1-m_j)/s_j)^2) = K0 * exp(-a*(x1+h)^2); K0
  drops out exactly under BN1 (scale invariance), h absorbs conv1_b.
- conv2_b drops out exactly under BN2 (shift invariance).

Device algorithm per core (out z-planes [8k, 8k+8), fp16 matmul path):
- conv1 as per-input-plane "contribution" matmuls: G[m=(dz,dy,dx), y', x'] =
  sum_c w1[c,m] * x[c, zp, y', x'] (K=32, M=27, fp16, fp32 PSUM), over padded
  66x66 planes; G cast to fp16 in SBUF.
- align-gather DMA assembles A27[m, y, x] = G[m, iy+dz, y+dy, x+dx]; the
  27-tap combine is a data-stationary matmul (lhsT=A27 chunk [27,128],
  rhs=ones) producing x1 directly in [128, n] layout.
- t = exp(-a*(x1+h)^2) on ScalarE; global sum/sumsq -> AllReduce -> s1,u1.
- y = s1*t+u1 (fp16) scattered into a padded DRAM volume; conv2 via im2col
  gather (K=27, M=32, fp16); per-channel sums -> AllReduce -> scale/shift;
  final affine + store.
"""

import numpy as np
import ml_dtypes

EPS = 1e-5
LAST_RES = None
DEVICE_OK = False
B, C, D, H, W = 4, 32, 64, 64, 64
NCORES = 8
ZP = D // NCORES            # 8 out planes per core
NZI = ZP + 4                # 12 input planes per core (2 halo each side)
NZY = ZP + 2                # 10 y planes per core (1 halo each side)
PW = W + 2                  # 66
PLANE = PW * PW             # 4356
OPLANE = H * W              # 4096
NTOT = float(B * D * H * W)  # 1048576
f16 = np.float16
_ = ml_dtypes  # (bf16 available if needed)


def _host_folds(conv1_w, conv1_b, conv2_w, mu, sigma):
    m = mu[0].astype(np.float64)
    s = sigma[0].astype(np.float64)
    a = float(np.sum(1.0 / s**2))
    bq = float(-2.0 * np.sum(m / s**2))
    h = bq / (2 * a) + float(conv1_b[0])
    w1 = conv1_w[0].astype(np.float32).reshape(C, 27)      # [32, 27]
    w2 = conv2_w[:, 0].astype(np.float32).reshape(C, 27).T  # [27, 32]
    return a, h, w1, w2


def _build(a, h, g1, b1):
    import concourse.bass as bass
    import concourse.tile as tile
    from concourse import mybir
    from contextlib import ExitStack

    dt = mybir.dt
    nc = bass.Bass(num_devices=NCORES)

    xin = nc.dram_tensor("xin", [C, B * NZI * PLANE], dt.float16, kind="ExternalInput")
    w1in = nc.dram_tensor("w1in", [C, 27], dt.float16, kind="ExternalInput")
    w2in = nc.dram_tensor("w2in", [27, C], dt.float16, kind="ExternalInput")
    selin = nc.dram_tensor("selin", [96, C], dt.float32, kind="ExternalInput")
    seltin = nc.dram_tensor("seltin", [C, 96], dt.float32, kind="ExternalInput")
    g2b2in = nc.dram_tensor("g2b2in", [C, 2], dt.float32, kind="ExternalInput")
    zmaskin = nc.dram_tensor("zmaskin", [128, 2], dt.float32, kind="ExternalInput")
    outd = nc.dram_tensor("out", [96, 24 * 2048], dt.float32, kind="ExternalOutput")
    dbg_x1 = nc.dram_tensor("dbg_x1", [128, B * NZY * 32], dt.float32, kind="ExternalOutput")
    dbg_t = nc.dram_tensor("dbg_t", [128, B * NZY * 32], dt.float32, kind="ExternalOutput")
    dbg_ar1 = nc.dram_tensor("dbg_ar1", [1, 2], dt.float32, kind="ExternalOutput")
    dbg_su = nc.dram_tensor("dbg_su", [1, 2], dt.float32, kind="ExternalOutput")
    dbg_yf = nc.dram_tensor("dbg_yf", [B * NZY, PLANE], dt.float16, kind="ExternalOutput")
    dbg_ar2 = nc.dram_tensor("dbg_ar2", [96, 2], dt.float32, kind="ExternalOutput")
    dbg_bc = nc.dram_tensor("dbg_bc", [96, 2], dt.float32, kind="ExternalOutput")

    CH1 = [(0, 1024), (1024, 1024), (2048, 1024), (3072, 1024), (4096, 260)]

    with tile.TileContext(nc) as tc:
        with ExitStack() as ctx:
            consts = ctx.enter_context(tc.tile_pool(name="consts", bufs=1))
            big = ctx.enter_context(tc.tile_pool(name="big", bufs=1))
            dram = ctx.enter_context(tc.tile_pool(name="dram", bufs=1, space="DRAM"))

            w1t = consts.tile([C, 27], dt.float16)
            w2t = consts.tile([27, C], dt.float16)
            selt = consts.tile([96, C], dt.float32)
            seltt = consts.tile([C, 96], dt.float32)
            g2b2 = consts.tile([C, 2], dt.float32)
            nc.sync.dma_start(out=w1t, in_=w1in[:, :])
            nc.sync.dma_start(out=w2t, in_=w2in[:, :])
            nc.sync.dma_start(out=selt, in_=selin[:, :])
            nc.sync.dma_start(out=seltt, in_=seltin[:, :])
            nc.sync.dma_start(out=g2b2, in_=g2b2in[:, :])
            zm = consts.tile([128, 2], dt.float32)
            nc.sync.dma_start(out=zm, in_=zmaskin[:, :])
            ones27 = consts.tile([27, 1], dt.float16)
            nc.vector.memset(ones27, 1.0)
            ones128 = consts.tile([128, 1], dt.float32)
            nc.vector.memset(ones128, 1.0)
            hB = consts.tile([128, 1], dt.float32)
            nc.vector.memset(hB, h)
            zB = consts.tile([128, 1], dt.float32)
            nc.vector.memset(zB, 0.0)
            eps1 = consts.tile([1, 1], dt.float32)
            nc.vector.memset(eps1, EPS)
            eps32 = consts.tile([C, 1], dt.float32)
            nc.vector.memset(eps32, EPS)
            ident = consts.tile([128, 128], dt.float16)
            from concourse.masks import make_identity
            make_identity(nc, ident)

            X1 = big.tile([128, B, NZY, 32], dt.float32)
            Tt = big.tile([128, B * NZY * 32], dt.float32)
            sq = big.tile([128, B * NZY * 32], dt.float32)
            Y16 = big.tile([128, B, NZY, 32], dt.float16)
            VS1 = big.tile([96, 24], dt.float32)
            VSQ = big.tile([96, 24], dt.float32)

            yflat = dram.tile([B * NZY, PLANE], dt.float16)
            b1i = dram.tile([1, 128], dt.float32)
            b1o = dram.tile([1, 128], dt.float32)
            b2i = dram.tile([96, 2], dt.float32)
            b2o = dram.tile([96, 2], dt.float32)
            su_d = dram.tile([1, 2], dt.float32)

            # ---------------- Phase 1: conv1 + exp + stats1 ----------------
            with tc.tile_pool(name="work1", bufs=1) as work1, \
                 tc.tile_pool(name="xtp", bufs=3) as xtp, \
                 tc.tile_pool(name="a27p", bufs=1) as a27p, \
                 tc.tile_pool(name="ps1", bufs=3, space="PSUM") as ps1, \
                 tc.tile_pool(name="ps1b", bufs=2, space="PSUM") as ps1b:
                Gsb = work1.tile([27, NZI, PLANE], dt.float16)
                Gv = Gsb.rearrange("p z (y x) -> p z y x", y=PW)
                for b in range(B):
                    for iz in range(NZI):
                        xt = xtp.tile([C, PLANE], dt.float16, tag="xt")
                        off = (b * NZI + iz) * PLANE
                        nc.sync.dma_start(out=xt, in_=xin[:, off:off + PLANE])
                        for ci, (c0, cl) in enumerate(CH1):
                            gp = ps1.tile([27, 1024], dt.float32, tag="gp")
                            for s0 in range(0, cl, 512):
                                sl = min(512, cl - s0)
                                nc.tensor.matmul(
                                    out=gp[:, s0:s0 + sl],
                                    lhsT=w1t,
                                    rhs=xt[:, c0 + s0:c0 + s0 + sl],
                                    start=True, stop=True,
                                )
                            if iz % 2 == 0:
                                nc.vector.tensor_copy(out=Gsb[:, iz, c0:c0 + cl],
                                                      in_=gp[:, 0:cl])
                            else:
                                nc.scalar.copy(out=Gsb[:, iz, c0:c0 + cl],
                                               in_=gp[:, 0:cl])
                    for r in range(2):
                        A27 = a27p.tile([27, 5, 64, 64], dt.float16, tag="a27")
                        for mm in range(27):
                            dz, dy, dx = mm // 9, (mm // 3) % 3, mm % 3
                            for p in range(5):
                                nc.sync.dma_start(
                                    out=A27[mm:mm + 1, p, :, :],
                                    in_=Gv[mm:mm + 1, 5 * r + dz + p,
                                           dy:dy + 64, dx:dx + 64],
                                )
                        A27f = A27.rearrange("p z y x -> p z (y x)")
                        for p in range(5):
                            iy = 5 * r + p
                            x1p = ps1b.tile([128, 32], dt.float32, tag="x1p")
                            for j in range(32):
                                nc.tensor.matmul(
                                    out=x1p[:, j:j + 1],
                                    lhsT=A27f[:, p, 128 * j:128 * (j + 1)],
                                    rhs=ones27,
                                    start=True, stop=True,
                                )
                            nc.vector.tensor_copy(out=X1[:, b, iy, :], in_=x1p)

            # exp + stats1
            X1v = X1.rearrange("p b z j -> p (b z j)")
            nc.scalar.activation(out=sq, in_=X1v,
                                 func=mybir.ActivationFunctionType.Square,
                                 bias=hB, scale=1.0)
            nc.scalar.activation(out=Tt, in_=sq,
                                 func=mybir.ActivationFunctionType.Exp,
                                 bias=zB, scale=-a)
            Tv = Tt.rearrange("p (b z j) -> p b z j", b=B, z=NZY)
            Town = Tv[:, :, 1:1 + ZP, :]
            p1 = big.tile([128, 1], dt.float32)
            p2 = big.tile([128, 1], dt.float32)
            nc.vector.tensor_reduce(out=p1, in_=Town,
                                    axis=mybir.AxisListType.XYZ,
                                    op=mybir.AluOpType.add)
            s2scr = sq[:, 0:1024].rearrange("p (b z j) -> p b z j", b=B, z=ZP)
            nc.vector.tensor_mul(s2scr, Town, Town)
            nc.vector.tensor_reduce(out=p2, in_=s2scr,
                                    axis=mybir.AxisListType.XYZ,
                                    op=mybir.AluOpType.add)
            Ssb = big.tile([1, 2], dt.float32)
            with tc.tile_pool(name="pst1", bufs=1, space="PSUM") as pst1:
                Sa = pst1.tile([1, 1], dt.float32)
                Sb = pst1.tile([1, 1], dt.float32)
                nc.tensor.matmul(out=Sa, lhsT=p1, rhs=ones128, start=True, stop=True)
                nc.tensor.matmul(out=Sb, lhsT=p2, rhs=ones128, start=True, stop=True)
                nc.vector.tensor_copy(out=Ssb[:, 0:1], in_=Sa)
                nc.vector.tensor_copy(out=Ssb[:, 1:2], in_=Sb)
            nc.sync.dma_start(out=b1i[0:1, 0:2], in_=Ssb)
            nc.gpsimd.collective_compute(
                "AllReduce", mybir.AluOpType.add,
                replica_groups=[list(range(NCORES))],
                ins=[b1i.opt()], outs=[b1o.opt()])
            ar1 = big.tile([1, 2], dt.float32)
            nc.sync.dma_start(out=ar1, in_=b1o[0:1, 0:2])

            # s1 = g1 / sqrt(var+eps); u1 = b1 - mean*s1
            m1 = big.tile([1, 1], dt.float32)
            q1 = big.tile([1, 1], dt.float32)
            v1 = big.tile([1, 1], dt.float32)
            r1 = big.tile([1, 1], dt.float32)
            s1 = big.tile([1, 1], dt.float32)
            u1 = big.tile([1, 1], dt.float32)
            nc.vector.tensor_scalar_mul(m1, ar1[:, 0:1], 1.0 / NTOT)
            nc.vector.tensor_scalar_mul(q1, ar1[:, 1:2], 1.0 / NTOT)
            nc.vector.tensor_mul(v1, m1, m1)
            nc.vector.tensor_sub(v1, q1, v1)
            nc.scalar.activation(out=v1, in_=v1,
                                 func=mybir.ActivationFunctionType.Sqrt,
                                 bias=eps1, scale=1.0)
            nc.vector.reciprocal(out=r1, in_=v1)
            nc.vector.tensor_scalar_mul(s1, r1, g1)
            nc.vector.tensor_mul(u1, m1, s1)
            nc.vector.tensor_scalar(u1, u1, -1.0, b1, op0=mybir.AluOpType.mult,
                                    op1=mybir.AluOpType.add)
            su = big.tile([1, 2], dt.float32)
            nc.vector.tensor_copy(out=su[:, 0:1], in_=s1)
            nc.vector.tensor_copy(out=su[:, 1:2], in_=u1)
            nc.sync.dma_start(out=su_d[0:1, 0:2], in_=su)
            suB = big.tile([128, 2], dt.float32)
            nc.sync.dma_start(out=suB, in_=su_d[0:1, 0:2].to_broadcast((128, 2)))

            nc.sync.dma_start(out=dbg_x1[:, :], in_=X1.rearrange("p b z j -> p (b z j)"))
            nc.sync.dma_start(out=dbg_t[:, :], in_=Tt)
            nc.sync.dma_start(out=dbg_ar1[:, :], in_=ar1)
            nc.sync.dma_start(out=dbg_su[:, :], in_=su)
            # ---------------- Phase 2: y, conv2, stats2 ----------------
            nc.scalar.activation(out=Y16.rearrange("p b z j -> p (b z j)"), in_=Tt,
                                 func=mybir.ActivationFunctionType.Identity,
                                 bias=suB[:, 1:2], scale=suB[:, 0:1])
            for b in range(B):
                nc.vector.tensor_scalar_mul(Y16[:, b, 0, :], Y16[:, b, 0, :],
                                            zm[:, 0:1])
                nc.vector.tensor_scalar_mul(Y16[:, b, NZY - 1, :],
                                            Y16[:, b, NZY - 1, :], zm[:, 1:2])
            zsb = big.tile([40, PLANE // 2], dt.float16)
            nc.vector.memset(zsb, 0.0)
            nc.sync.dma_start(out=yflat[:, 0:PLANE // 2], in_=zsb)
            nc.sync.dma_start(out=yflat[:, PLANE // 2:PLANE], in_=zsb)
            yfv = yflat.rearrange("r (y x) -> r y x", y=PW)
            with tc.tile_pool(name="psy", bufs=2, space="PSUM") as psy, \
                 tc.tile_pool(name="ytp", bufs=2) as ytp:
                for b in range(B):
                    for iy in range(NZY):
                        row = b * NZY + iy
                        ytP = psy.tile([32, 128], dt.float16, tag="ytP")
                        nc.tensor.transpose(ytP, Y16[:, b, iy, :], ident)
                        yts = ytp.tile([32, 128], dt.float16, tag="yts")
                        nc.vector.tensor_copy(out=yts, in_=ytP)
                        dst = yfv[row:row + 1, 1:65, 1:65].rearrange(
                            "r (y2 p2) x -> r y2 p2 x", p2=2)
                        nc.sync.dma_start(
                            out=dst[0, :, :, :],
                            in_=yts.rearrange("j (p2 x) -> j p2 x", p2=2))

            nc.sync.dma_start(out=dbg_yf[:, :], in_=yflat[:, :])
            with tc.tile_pool(name="work2", bufs=1) as work2, \
                 tc.tile_pool(name="rp", bufs=1) as rp, \
                 tc.tile_pool(name="outp", bufs=2) as outp:
                Vsb = work2.tile([96, 24 * 2048], dt.float16)
                vscr = work2.tile([96, 2048], dt.float16)
                ps2_cm = tc.tile_pool(name="ps2", bufs=2, space="PSUM")
                ps2 = ps2_cm.__enter__()
                for b in range(B):
                    R = rp.tile([27, ZP, 64, 64], dt.float16, tag="r")
                    Rf = R.rearrange("p z y x -> p z (y x)")
                    yfv1 = yflat.rearrange("(o r) (y x) -> o r y x", o=1, y=PW)
                    for mm in range(27):
                        dz, dy, dx = mm // 9, (mm // 3) % 3, mm % 3
                        for oz in range(ZP):
                            nc.sync.dma_start(
                                out=R[mm:mm + 1, oz, :, :],
                                in_=yfv1[:, b * NZY + dz + oz,
                                         dy:dy + 64, dx:dx + 64],
                            )
                    for gl in range(6):          # pack index within b (3 lanes)
                        g = b * 6 + gl
                        nlane = 3 if gl < 5 else 1
                        Vp = ps2.tile([96, 2048], dt.float32, tag="vp")
                        for lane in range(nlane):
                            hp = gl * 3 + lane   # within-b half-plane index
                            oz, half = hp // 2, hp % 2
                            for s0 in range(0, 2048, 512):
                                nc.tensor.matmul(
                                    out=Vp[lane * 32:(lane + 1) * 32, s0:s0 + 512],
                                    lhsT=w2t,
                                    rhs=Rf[:, oz, half * 2048 + s0:half * 2048 + s0 + 512],
                                    start=True, stop=True,
                                )
                        np_ = nlane * 32
                        nc.scalar.activation(
                            out=Vsb[0:np_, g * 2048:(g + 1) * 2048], in_=Vp[0:np_, :],
                            func=mybir.ActivationFunctionType.Copy,
                            bias=0.0, scale=1.0, accum_out=VS1[0:np_, g:g + 1])
                        nc.vector.tensor_mul(
                            vscr[0:np_, :], Vsb[0:np_, g * 2048:(g + 1) * 2048],
                            Vsb[0:np_, g * 2048:(g + 1) * 2048])
                        nc.vector.tensor_reduce(
                            out=VSQ[0:np_, g:g + 1], in_=vscr[0:np_, :],
                            axis=mybir.AxisListType.X,
                            op=mybir.AluOpType.add)
                        if nlane < 3:
                            nc.vector.memset(VS1[32:64, g:g + 1], 0.0)
                            nc.vector.memset(VS1[64:96, g:g + 1], 0.0)
                            nc.vector.memset(VSQ[32:64, g:g + 1], 0.0)
                            nc.vector.memset(VSQ[64:96, g:g + 1], 0.0)

                ps2_cm.__exit__(None, None, None)
                vs1 = big.tile([96, 1], dt.float32)
                vsq = big.tile([96, 1], dt.float32)
                nc.vector.tensor_reduce(out=vs1, in_=VS1,
                                        axis=mybir.AxisListType.X,
                                        op=mybir.AluOpType.add)
                nc.vector.tensor_reduce(out=vsq, in_=VSQ,
                                        axis=mybir.AxisListType.X,
                                        op=mybir.AluOpType.add)
                B2 = big.tile([96, 2], dt.float32)
                nc.vector.tensor_copy(out=B2[:, 0:1], in_=vs1)
                nc.vector.tensor_copy(out=B2[:, 1:2], in_=vsq)
                nc.sync.dma_start(out=b2i[:, :], in_=B2)
                nc.gpsimd.collective_compute(
                    "AllReduce", mybir.AluOpType.add,
                    replica_groups=[list(range(NCORES))],
                    ins=[b2i.opt()], outs=[b2o.opt()])
                ar2 = big.tile([96, 2], dt.float32)
                nc.sync.dma_start(out=ar2, in_=b2o[:, :])

                nc.sync.dma_start(out=dbg_ar2[:, :], in_=ar2)
                with tc.tile_pool(name="pst2", bufs=1, space="PSUM") as pst2:
                    S32 = pst2.tile([C, 2], dt.float32)
                    nc.tensor.matmul(out=S32, lhsT=selt, rhs=ar2,
                                     start=True, stop=True)
                    ss = big.tile([C, 2], dt.float32)
                    nc.vector.tensor_copy(out=ss, in_=S32)
                    mean2 = big.tile([C, 1], dt.float32)
                    q2 = big.tile([C, 1], dt.float32)
                    v2 = big.tile([C, 1], dt.float32)
                    rc2 = big.tile([C, 1], dt.float32)
                    sc2 = big.tile([C, 1], dt.float32)
                    sh2 = big.tile([C, 1], dt.float32)
                    nc.vector.tensor_scalar_mul(mean2, ss[:, 0:1], 1.0 / NTOT)
                    nc.vector.tensor_scalar_mul(q2, ss[:, 1:2], 1.0 / NTOT)
                    nc.vector.tensor_mul(v2, mean2, mean2)
                    nc.vector.tensor_sub(v2, q2, v2)
                    nc.scalar.activation(out=v2, in_=v2,
                                         func=mybir.ActivationFunctionType.Sqrt,
                                         bias=eps32, scale=1.0)
                    nc.vector.reciprocal(out=rc2, in_=v2)
                    nc.vector.tensor_mul(sc2, rc2, g2b2[:, 0:1])
                    nc.vector.tensor_mul(sh2, mean2, sc2)
                    nc.vector.tensor_sub(sh2, g2b2[:, 1:2], sh2)
                    scsh = big.tile([C, 2], dt.float32)
                    nc.vector.tensor_copy(out=scsh[:, 0:1], in_=sc2)
                    nc.vector.tensor_copy(out=scsh[:, 1:2], in_=sh2)
                    BCp = pst2.tile([96, 2], dt.float32)
                    nc.tensor.matmul(out=BCp, lhsT=seltt, rhs=scsh,
                                     start=True, stop=True)
                    bcB = big.tile([96, 2], dt.float32)
                    nc.vector.tensor_copy(out=bcB, in_=BCp)
                    nc.sync.dma_start(out=dbg_bc[:, :], in_=bcB)

                for g in range(24):
                    OUTt = outp.tile([96, 2048], dt.float32, tag="outt")
                    nc.scalar.activation(
                        out=OUTt, in_=Vsb[:, g * 2048:(g + 1) * 2048],
                        func=mybir.ActivationFunctionType.Identity,
                        bias=bcB[:, 1:2], scale=bcB[:, 0:1])
                    nc.sync.dma_start(out=outd[:, g * 2048:(g + 1) * 2048],
                                      in_=OUTt)

    from waitfix_inline import fix_multiwaits
    fix_multiwaits(nc)
    return nc


def _run_device(x, conv1_w, conv1_b, conv2_w, mu, sigma, bn1_g, bn1_b,
                bn2_g, bn2_b):
    from concourse.bass_utils import run_bass_kernel_spmd

    a, h, w1, w2 = _host_folds(conv1_w, conv1_b, conv2_w, mu, sigma)
    g1, b1 = float(bn1_g[0]), float(bn1_b[0])
    nc = _build(a, h, g1, b1)

    w1t = w1.astype(f16)
    w2t = np.ascontiguousarray(w2).astype(f16)
    sel = np.zeros((96, C), np.float32)
    sel[np.arange(96), np.arange(96) % C] = 1.0
    selt = np.ascontiguousarray(sel.T)
    g2b2 = np.ascontiguousarray(
        np.stack([bn2_g.astype(np.float32), bn2_b.astype(np.float32)], axis=1))

    xb = x.astype(f16)
    xpad = np.zeros((B, C, D + 4, PW, PW), dtype=f16)
    xpad[:, :, 2:-2, 1:-1, 1:-1] = xb
    in_maps = []
    for k in range(NCORES):
        sh = xpad[:, :, ZP * k:ZP * k + NZI]          # [B, C, 12, 66, 66]
        sh = np.ascontiguousarray(sh.transpose(1, 0, 2, 3, 4)).reshape(
            C, B * NZI * PLANE)
        zmask = np.ones((128, 2), np.float32)
        if k == 0:
            zmask[:, 0] = 0
        if k == NCORES - 1:
            zmask[:, 1] = 0
        in_maps.append({"xin": sh, "w1in": w1t, "w2in": w2t, "selin": sel,
                        "seltin": selt, "g2b2in": g2b2, "zmaskin": zmask})

    import os
    trace = bool(os.environ.get("BASS_TRACE"))
    res = run_bass_kernel_spmd(nc, in_maps, core_ids=list(range(NCORES)),
                               trace=trace)
    global LAST_RES
    LAST_RES = res

    out = np.empty((B, C, D, H, W), np.float32)
    for k in range(NCORES):
        O = res.results[k]["out"].reshape(3, C, 24, 32, 64)
        for b_ in range(B):
            for hpl in range(16):
                gl, lane = hpl // 3, hpl % 3
                g = b_ * 6 + gl
                oz, half = hpl // 2, hpl % 2
                out[b_, :, ZP * k + oz, half * 32:(half + 1) * 32, :] = \
                    O[lane, :, g]
    return out


def _numpy_fallback(x, conv1_w, conv1_b, conv2_w, conv2_b, mu, sigma,
                    bn1_g, bn1_b, bn2_g, bn2_b):
    def conv3d(xx, w):
        b_, ci, d_, h_, wd = xx.shape
        o = w.shape[0]
        xp = np.zeros((b_, ci, d_ + 2, h_ + 2, wd + 2), np.float32)
        xp[:, :, 1:-1, 1:-1, 1:-1] = xx
        out = np.zeros((b_, o, d_, h_, wd), np.float32)
        for dz in range(3):
            for dy in range(3):
                for dx in range(3):
                    out += np.einsum(
                        "oc,bczyx->bozyx", w[:, :, dz, dy, dx],
                        xp[:, :, dz:dz + d_, dy:dy + h_, dx:dx + wd],
                        optimize=True)
        return out

    x = x.astype(np.float32)
    x1 = conv3d(x, conv1_w.astype(np.float32)) \
        + conv1_b.astype(np.float32)[None, :, None, None, None]
    m = mu[0].astype(np.float64)
    s = sigma[0].astype(np.float64)
    aa = np.sum(1.0 / s**2)
    bb = -2.0 * np.sum(m / s**2)
    cc = np.sum(m**2 / s**2)
    x1d = x1.astype(np.float64)
    fuzz = np.exp(-(aa * x1d * x1d + bb * x1d + cc)).astype(np.float32)
    mean1 = fuzz.mean(dtype=np.float64)
    var1 = np.mean((fuzz.astype(np.float64) - mean1) ** 2)
    y = ((fuzz - mean1) / np.sqrt(var1 + EPS)).astype(np.float32)
    y = bn1_g.astype(np.float32)[0] * y + bn1_b.astype(np.float32)[0]
    v = conv3d(y, conv2_w.astype(np.float32)) \
        + conv2_b.astype(np.float32)[None, :, None, None, None]
    vd = v.astype(np.float64)
    mean2 = vd.mean(axis=(0, 2, 3, 4))
    var2 = ((vd - mean2[None, :, None, None, None]) ** 2).mean(axis=(0, 2, 3, 4))
    sc = bn2_g.astype(np.float64) / np.sqrt(var2 + EPS)
    sh = bn2_b.astype(np.float64) - mean2 * sc
    return (vd * sc[None, :, None, None, None]
            + sh[None, :, None, None, None]).astype(np.float32)


def kernel(x, conv1_w, conv1_b, conv2_w, conv2_b, mu, sigma,
           bn1_g, bn1_b, bn2_g, bn2_b):
    x = np.asarray(x, np.float32)
    args = dict(
        x=x,
        conv1_w=np.asarray(conv1_w, np.float32),
        conv1_b=np.asarray(conv1_b, np.float32),
        conv2_w=np.asarray(conv2_w, np.float32),
        mu=np.asarray(mu, np.float32), sigma=np.asarray(sigma, np.float32),
        bn1_g=np.asarray(bn1_g, np.float32), bn1_b=np.asarray(bn1_b, np.float32),
        bn2_g=np.asarray(bn2_g, np.float32), bn2_b=np.asarray(bn2_b, np.float32),
    )
    try:
        out = _run_device(**args)
        global DEVICE_OK
        DEVICE_OK = True
        return out
    except Exception:
        import traceback
        traceback.print_exc()
        return _numpy_fallback(
            x, args["conv1_w"], args["conv1_b"], args["conv2_w"],
            np.asarray(conv2_b, np.float32), args["mu"], args["sigma"],
            args["bn1_g"], args["bn1_b"], args["bn2_g"], args["bn2_b"])


# ---- inline waitfix module (kernel.py must be self-contained) ----
import sys as _sys
import types as _types

_wf = _types.ModuleType("waitfix_inline")
_wf_code = '''
from concourse import mybir
_ctr = [0]
def fix_multiwaits(nc, keep_embedded=1):
    n_split = 0
    for f in nc.m.functions:
        for bb in f.blocks:
            out = []
            changed = False
            for inst in bb.instructions:
                si = inst.sync_info
                waits = list(si.on_wait) if si is not None else []
                if len(waits) > keep_embedded:
                    extra, keep = waits[:-keep_embedded], waits[-keep_embedded:]
                    for w in extra:
                        _ctr[0] += 1
                        ev = mybir.InstEventSemaphore(
                            name="waitsplit_%d" % _ctr[0],
                            ins=[], outs=[], engine=inst.engine,
                            sync_info=mybir.SyncInfo(on_wait=[w], on_update=[]),
                        )
                        out.append(ev)
                    inst.sync_info = mybir.SyncInfo(
                        on_wait=keep, on_update=list(si.on_update))
                    n_split += 1
                    changed = True
                out.append(inst)
            if changed:
                bb.instructions = out
    return n_split
'''
exec(_wf_code, _wf.__dict__)
_sys.modules["waitfix_inline"] = _wf



# revision 8
# speedup vs baseline: 1.7300x; 1.7300x over previous
"""Trainium2 kernel v2 for nn_BCE_35270271435550.

Sharding: core k = (b, h) with b = k//2, h = k%2; each core owns output planes
z in [32h, 32h+32) of batch b. x1/t computed on rows r=1..34 (z = 32h-2+r).

Per-core pipeline (all fp16 matmuls, f32 PSUM):
- G-matmul: 4-plane-packed conv1 contribution G[(m,q), px] via block-diag W4
  (K=128, M=108, rows 27q+m), drained to Gtmp then DMA-reshuffled into a
  14-slot plane-major ring Gw [27, 14*4490] (slot = zi mod 14, data at +67).
- combine: per 12-plane group G12 (bases r=1,12,23), per dz: A9 [108=12*e9+g,
  4356] gathered from Gw rows m with per-(dy,dx) window shifts; 3 accumulating
  matmuls (lhsT=A9 chunk [108,121], rhs=block-ones OB [108,12]) -> x1 [121,12].
- exp path on x1pix [121, 1296] f32: Square(bias=h), Exp(scale=-a) -> tpix f16.
- transpose per plane r: tpix col-slice [121,36] -> [36,121] PSUM -> stg ->
  fold-DMA into tflat[r, 67:4423]. Borders zeroed; bn_stats for BN1 stats.
- AllReduce 1 -> s1,u1; y-affine in-place on tflat interior with zm row mask.
- conv2: per out-group jp: R4 [108=27g+m, 4356] gathered from tflat rows with
  window shifts (custom overlapped-stride src APs); matmuls lhsT=W2B [108,128]
  -> V [128=32g+ch, 484-chunks]; scalar drains -> Vsb f16.
- BN2 sums via DVE reduce + tensor_tensor_reduce; AllReduce 2 -> sc,sh;
  final affine on DVE -> outd [128, 8*4356] f16; host strips pads/reassembles.
"""

import numpy as np

import os as _os
EPS = 1e-5
LAST_RES = None
DEVICE_OK = False
DEBUG = bool(_os.environ.get("K2_DEBUG"))
PHASE1_ONLY = bool(_os.environ.get("K2_PHASE1"))
NOAR = bool(_os.environ.get("K2_NOAR"))
B, C, D, H, W = 4, 32, 64, 64, 64
NCORES = 8
RW = 4490            # padded row: 66*66 + 2*67
PLANE = 4356         # 66*66
NSLOT = 14           # Gw ring slots
NTOT = float(B * D * H * W)  # 1048576
f16 = np.float16


def _host_folds(conv1_w, conv1_b, conv2_w, mu, sigma):
    m = mu[0].astype(np.float64)
    s = sigma[0].astype(np.float64)
    a = float(np.sum(1.0 / s**2))
    bq = float(-2.0 * np.sum(m / s**2))
    h = bq / (2 * a) + float(conv1_b[0])
    w1 = conv1_w[0].astype(np.float32).reshape(C, 27)       # [32, 27]
    w2 = conv2_w[:, 0].astype(np.float32).reshape(C, 27).T  # [27, 32]
    return a, h, w1, w2


def _build(a, h, g1, b1):
    import concourse.bass as bass
    import concourse.tile as tile
    from concourse import mybir
    from concourse.ap import AP
    from concourse.masks import make_identity
    from contextlib import ExitStack

    dt = mybir.dt
    AF = mybir.ActivationFunctionType
    ALU = mybir.AluOpType
    AX = mybir.AxisListType
    nc = bass.Bass(num_devices=NCORES)

    xin = nc.dram_tensor("xin", [128, 9 * PLANE], dt.float16, kind="ExternalInput")
    w4in = nc.dram_tensor("w4in", [128, 108], dt.float16, kind="ExternalInput")
    obin = nc.dram_tensor("obin", [108, 12], dt.float16, kind="ExternalInput")
    w2in = nc.dram_tensor("w2in", [108, 128], dt.float16, kind="ExternalInput")
    sel1in = nc.dram_tensor("sel1in", [128, 32], dt.float32, kind="ExternalInput")
    sel2in = nc.dram_tensor("sel2in", [32, 128], dt.float32, kind="ExternalInput")
    zmin = nc.dram_tensor("zmin", [36, 1], dt.float32, kind="ExternalInput")
    mrin = nc.dram_tensor("mrin", [36, 1], dt.float32, kind="ExternalInput")
    g2b2in = nc.dram_tensor("g2b2in", [32, 2], dt.float32, kind="ExternalInput")
    outd = nc.dram_tensor("out", [128, 8 * PLANE], dt.float16, kind="ExternalOutput")
    if DEBUG:
        dbg_tflat = nc.dram_tensor("dbg_tflat", [36, RW], dt.float16, kind="ExternalOutput")
        dbg_x1 = nc.dram_tensor("dbg_x1", [121, 1296], dt.float32, kind="ExternalOutput")
        dbg_ar1 = nc.dram_tensor("dbg_ar1", [1, 2], dt.float32, kind="ExternalOutput")
        dbg_su = nc.dram_tensor("dbg_su", [1, 2], dt.float32, kind="ExternalOutput")
        dbg_s2 = nc.dram_tensor("dbg_s2", [128, 2], dt.float32, kind="ExternalOutput")
        dbg_bc = nc.dram_tensor("dbg_bc", [128, 2], dt.float32, kind="ExternalOutput")
        dbg_yf = nc.dram_tensor("dbg_yf", [36, RW], dt.float16, kind="ExternalOutput")

    with tile.TileContext(nc) as tc:
        with ExitStack() as ctx:
            consts = ctx.enter_context(tc.tile_pool(name="consts", bufs=1))
            shared = ctx.enter_context(tc.tile_pool(name="shared", bufs=1))
            dram = ctx.enter_context(tc.tile_pool(name="dram", bufs=1, space="DRAM"))

            W4 = consts.tile([128, 108], dt.float16)
            OB = consts.tile([108, 12], dt.float16)
            W2B = consts.tile([108, 128], dt.float16)
            sel1 = consts.tile([128, 32], dt.float32)
            sel2 = consts.tile([32, 128], dt.float32)
            zm = consts.tile([36, 1], dt.float32)
            mrows = consts.tile([36, 1], dt.float32)
            g2b2 = consts.tile([32, 2], dt.float32)
            nc.sync.dma_start(out=W4, in_=w4in[:, :])
            nc.sync.dma_start(out=OB, in_=obin[:, :])
            nc.sync.dma_start(out=W2B, in_=w2in[:, :])
            nc.sync.dma_start(out=sel1, in_=sel1in[:, :])
            nc.sync.dma_start(out=sel2, in_=sel2in[:, :])
            nc.sync.dma_start(out=zm, in_=zmin[:, :])
            nc.sync.dma_start(out=mrows, in_=mrin[:, :])
            nc.sync.dma_start(out=g2b2, in_=g2b2in[:, :])
            ident = consts.tile([128, 128], dt.float16)
            make_identity(nc, ident)
            hB = consts.tile([121, 1], dt.float32)
            nc.vector.memset(hB, h)
            eps1 = consts.tile([1, 1], dt.float32)
            nc.vector.memset(eps1, EPS)
            eps32 = consts.tile([32, 1], dt.float32)
            nc.vector.memset(eps32, EPS)

            tflat = shared.tile([36, RW], dt.float16)
            x1pix = shared.tile([121, 1296], dt.float32)
            tpix = shared.tile([121, 1296], dt.float16)
            bnout = shared.tile([36, 54], dt.float32)
            bnagg = shared.tile([36, 2], dt.float32)
            rowstats = shared.tile([36, 2], dt.float32)
            Ssb = shared.tile([1, 2], dt.float32)
            ar1 = shared.tile([1, 2], dt.float32)
            su = shared.tile([1, 2], dt.float32)
            suB36 = shared.tile([36, 2], dt.float32)
            szm = shared.tile([36, 1], dt.float32)
            uzm = shared.tile([36, 1], dt.float32)
            VBN = shared.tile([128, 8 * 54], dt.float32)
            VAG = shared.tile([128, 2], dt.float32)
            S2 = shared.tile([128, 2], dt.float32)
            ar2 = shared.tile([128, 2], dt.float32)
            bcB = shared.tile([128, 2], dt.float32)

            yfd = dram.tile([36, RW], dt.float16)
            b1i = dram.tile([1, 2], dt.float32)
            b1o = dram.tile([1, 2], dt.float32)
            su_d = dram.tile([1, 2], dt.float32)
            b2i = dram.tile([128, 2], dt.float32)
            b2o = dram.tile([128, 2], dt.float32)

            nc.vector.memset(tflat[:, :], 0.0)

            # ---------------- Phase 1: conv1 -> x1 -> t ----------------
            with tc.tile_pool(name="p1", bufs=1) as p1, \
                 tc.tile_pool(name="xp", bufs=2) as xp, \
                 tc.tile_pool(name="gtp", bufs=2) as gtp, \
                 tc.tile_pool(name="a9p", bufs=3) as a9p, \
                 tc.tile_pool(name="stgp", bufs=2) as stgp, \
                 tc.tile_pool(name="pg", bufs=3, space="PSUM") as pg, \
                 tc.tile_pool(name="px1", bufs=2, space="PSUM") as px1, \
                 tc.tile_pool(name="ptr", bufs=2, space="PSUM") as ptr:
                Gw = p1.tile([27, NSLOT * RW], dt.float16)
                # zero the per-slot 67-pads once (junk there only feeds pad
                # pixels, but keep it clean)
                gwb = Gw[:, :]
                nc.vector.memset(
                    AP(tensor=gwb.tensor, offset=gwb.offset,
                       ap=[[NSLOT * RW, 27], [RW, NSLOT], [1, 67]]), 0.0)
                nc.vector.memset(
                    AP(tensor=gwb.tensor, offset=gwb.offset + 67 + PLANE,
                       ap=[[NSLOT * RW, 27], [RW, NSLOT], [1, 67]]), 0.0)

                gtmp_tiles = {}

                def emit_resh(gi, q):
                    zi = 4 * gi + q
                    slot = zi % NSLOT
                    nc.scalar.dma_start(
                        out=Gw[:, slot * RW + 67: slot * RW + 67 + PLANE],
                        in_=gtmp_tiles[gi][27 * q: 27 * (q + 1), :])

                def emit_ggroup(gi, qs=(0, 1, 2, 3)):
                    X = xp.tile([128, PLANE], dt.float16, tag="x")
                    nc.sync.dma_start(out=X, in_=xin[:, gi * PLANE:(gi + 1) * PLANE])
                    Gt = gtp.tile([108, PLANE], dt.float16, tag="gt")
                    gtmp_tiles[gi] = Gt
                    for ci in range(9):
                        gp = pg.tile([108, 484], dt.float32, tag="gp")
                        nc.tensor.matmul(out=gp, lhsT=W4,
                                         rhs=X[:, ci * 484:(ci + 1) * 484],
                                         start=True, stop=True)
                        if ci % 2 == 0:
                            nc.scalar.copy(out=Gt[:, ci * 484:(ci + 1) * 484], in_=gp)
                        else:
                            nc.vector.tensor_copy(out=Gt[:, ci * 484:(ci + 1) * 484], in_=gp)
                    for q in qs:
                        emit_resh(gi, q)

                def emit_combine(i12, b12):
                    A9s = []
                    for dz in range(3):
                        A9 = a9p.tile([108, PLANE], dt.float16, tag="a9")
                        A9s.append(A9)
                        for dy in range(3):
                            for dx in range(3):
                                m = 9 * dz + 3 * dy + dx
                                e9 = 3 * dy + dx
                                off = 66 * dy + dx
                                g = 0
                                while g < 12:
                                    zi = b12 + g + dz - 1
                                    s0 = zi % NSLOT
                                    glen = min(NSLOT - s0, 12 - g)
                                    src = Gw[m:m + 1,
                                             s0 * RW + off: (s0 + glen - 1) * RW + off + PLANE]
                                    sap = AP(tensor=src.tensor, offset=src.offset,
                                             ap=[[NSLOT * RW, 1], [RW, glen], [1, PLANE]])
                                    nc.scalar.dma_start(
                                        out=A9[12 * e9 + g: 12 * e9 + g + glen, :],
                                        in_=sap)
                                    g += glen
                    for ci in range(36):
                        xo = px1.tile([121, 12], dt.float32, tag="xo")
                        for dz in range(3):
                            nc.tensor.matmul(
                                out=xo, lhsT=A9s[dz][:, ci * 121:(ci + 1) * 121],
                                rhs=OB, start=(dz == 0), stop=(dz == 2))
                        nc.vector.tensor_copy(
                            out=x1pix[:, (i12 * 36 + ci) * 12:(i12 * 36 + ci) * 12 + 12],
                            in_=xo)

                # G production interleaved with combines. Reshuffles of planes
                # that would clobber ring slots still needed by the pending
                # combine are deferred until after it.
                for gi in range(3):
                    emit_ggroup(gi)
                emit_ggroup(3, qs=(0, 1))          # planes 14,15 deferred
                emit_combine(0, 1)                  # needs planes 0..13
                emit_resh(3, 2)
                emit_resh(3, 3)
                emit_ggroup(4)
                emit_ggroup(5)
                emit_ggroup(6, qs=(0,))            # planes 25,26,27 deferred
                emit_combine(1, 12)                 # needs planes 11..24
                emit_resh(6, 1)
                emit_resh(6, 2)
                emit_resh(6, 3)
                emit_ggroup(7)
                emit_ggroup(8)
                emit_combine(2, 23)                 # needs planes 22..35

                # exp path: sq in-place (f32), then exp -> tpix (f16)
                nc.scalar.activation(out=x1pix[:, :], in_=x1pix[:, :],
                                     func=AF.Square, bias=hB, scale=1.0)
                if DEBUG:
                    nc.sync.dma_start(out=dbg_x1[:, :], in_=x1pix[:, :])
                nc.scalar.activation(out=tpix[:, :], in_=x1pix[:, :],
                                     func=AF.Exp, bias=0.0, scale=-a)

                # transpose per plane r=1..34 into tflat rows
                tpv = tpix[:, :].rearrange("p (blk g) -> p blk g", g=12)
                for r in range(1, 35):
                    i12 = 0 if r <= 12 else (1 if r <= 23 else 2)
                    b12 = (1, 12, 23)[i12]
                    g = r - b12
                    trp = ptr.tile([36, 121], dt.float16, tag="tr")
                    nc.tensor.transpose(
                        trp, tpv[:, i12 * 36:(i12 + 1) * 36, g], ident[0:121, 0:121])
                    stg = stgp.tile([36, 121], dt.float16, tag="stg")
                    nc.vector.tensor_copy(out=stg, in_=trp)
                    nc.gpsimd.dma_start(
                        out=tflat[r:r + 1, 67:67 + PLANE].rearrange(
                            "o (c px) -> o c px", c=36),
                        in_=stg[:, :])

            # zero plane borders inside the window: y=0 row, y=65 row, x cols
            nc.vector.memset(tflat[:, 67:133], 0.0)
            nc.vector.memset(tflat[:, 67 + 65 * 66:67 + 66 * 66], 0.0)
            tfv = tflat[:, 133:133 + 64 * 66].rearrange("r (y x) -> r y x", x=66)
            nc.vector.memset(tfv[:, :, 0:1], 0.0)
            nc.vector.memset(tfv[:, :, 65:66], 0.0)

            # BN1 stats over own rows (mask via mrows in the fold matmul)
            for ci in range(9):
                nc.vector.bn_stats(
                    out=bnout[:, ci * 6:(ci + 1) * 6],
                    in_=tflat[:, 67 + ci * 484:67 + (ci + 1) * 484])
            nc.vector.bn_aggr(out=bnagg[:, :], in_=bnout[:, :])
            tmp1 = shared.tile([36, 1], dt.float32)
            nc.vector.tensor_scalar_mul(rowstats[:, 0:1], bnagg[:, 0:1], float(PLANE))
            nc.vector.tensor_mul(tmp1, bnagg[:, 0:1], bnagg[:, 0:1])
            nc.vector.tensor_add(tmp1, bnagg[:, 1:2], tmp1)
            nc.vector.tensor_scalar_mul(rowstats[:, 1:2], tmp1, float(PLANE))
            with tc.tile_pool(name="pst1", bufs=1, space="PSUM") as pst1:
                Sp = pst1.tile([1, 2], dt.float32)
                nc.tensor.matmul(out=Sp, lhsT=mrows, rhs=rowstats,
                                 start=True, stop=True)
                nc.vector.tensor_copy(out=Ssb, in_=Sp)
            nc.sync.dma_start(out=b1i[0:1, 0:2], in_=Ssb)
            if NOAR:
                nc.sync.dma_start(out=b1o[0:1, 0:2], in_=b1i[0:1, 0:2])
            else:
                nc.gpsimd.collective_compute(
                    "AllReduce", mybir.AluOpType.add,
                    replica_groups=[list(range(NCORES))],
                    ins=[b1i.opt()], outs=[b1o.opt()])
            nc.sync.dma_start(out=ar1, in_=b1o[0:1, 0:2])

            # s1 = g1 / sqrt(var+eps); u1 = b1 - mean*s1
            m1 = shared.tile([1, 1], dt.float32)
            q1 = shared.tile([1, 1], dt.float32)
            v1 = shared.tile([1, 1], dt.float32)
            r1 = shared.tile([1, 1], dt.float32)
            s1 = shared.tile([1, 1], dt.float32)
            u1 = shared.tile([1, 1], dt.float32)
            nc.vector.tensor_scalar_mul(m1, ar1[:, 0:1], 1.0 / NTOT)
            nc.vector.tensor_scalar_mul(q1, ar1[:, 1:2], 1.0 / NTOT)
            nc.vector.tensor_mul(v1, m1, m1)
            nc.vector.tensor_sub(v1, q1, v1)
            nc.scalar.activation(out=v1, in_=v1, func=AF.Sqrt, bias=eps1, scale=1.0)
            nc.vector.reciprocal(out=r1, in_=v1)
            nc.vector.tensor_scalar_mul(s1, r1, g1)
            nc.vector.tensor_mul(u1, m1, s1)
            nc.vector.tensor_scalar(u1, u1, -1.0, b1, op0=ALU.mult, op1=ALU.add)
            nc.vector.tensor_copy(out=su[:, 0:1], in_=s1)
            nc.vector.tensor_copy(out=su[:, 1:2], in_=u1)
            nc.sync.dma_start(out=su_d[0:1, 0:2], in_=su)
            nc.sync.dma_start(out=suB36, in_=su_d[0:1, 0:2].to_broadcast((36, 2)))
            if DEBUG:
                nc.sync.dma_start(out=dbg_tflat[:, :], in_=tflat[:, :])
                nc.sync.dma_start(out=dbg_ar1[:, :], in_=ar1)
                nc.sync.dma_start(out=dbg_su[:, :], in_=su)
            nc.vector.tensor_mul(szm, suB36[:, 0:1], zm)
            nc.vector.tensor_mul(uzm, suB36[:, 1:2], zm)
            # y-affine in place on interior (borders stay 0): window rows
            # yy=1..64 live at 67 + 66*yy + xx, xx interior 1..64
            tin = tflat[:, 133:133 + 64 * 66].rearrange(
                "r (y x) -> r y x", x=66)[:, :, 1:65]
            nc.vector.tensor_scalar(tin, tin, szm, uzm, op0=ALU.mult, op1=ALU.add)
            # stage y to DRAM so R4 im2col gathers can use flat 3-dim src APs
            nc.sync.dma_start(out=yfd[:, :], in_=tflat[:, :])
            if DEBUG:
                nc.sync.dma_start(out=dbg_yf[:, :], in_=tflat[:, :])

            # ---------------- Phase 2: conv2 + BN2 ----------------
            with tc.tile_pool(name="rp", bufs=2) as rp, \
                 tc.tile_pool(name="p2", bufs=1) as p2, \
                 tc.tile_pool(name="op", bufs=2) as op, \
                 tc.tile_pool(name="pv", bufs=6, space="PSUM") as pv, \
                 tc.tile_pool(name="pst2", bufs=1, space="PSUM") as pst2:
                Vsb = p2.tile([128, 8 * PLANE], dt.float16)
                yfb = yfd[:, :]
                for jp in range(8):
                    R4 = rp.tile([108, PLANE], dt.float16, tag="r4")
                    R4v = R4[:, :].rearrange("(g m) px -> g m px", g=4)
                    for g in range(4):
                        for dz in range(3):
                            r = 4 * jp + g + dz + 1
                            sap = AP(tensor=yfb.tensor, offset=yfb.offset + r * RW,
                                     ap=[[66, 3], [1, 3], [1, PLANE]])
                            dap = R4v[g, 9 * dz:9 * dz + 9, :]
                            eng = nc.sync if (g + dz) % 2 == 0 else nc.scalar
                            eng.dma_start(out=dap, in_=sap)
                    for ci in range(9):
                        vp = pv.tile([128, 484], dt.float32, tag="vp")
                        nc.tensor.matmul(out=vp, lhsT=W2B,
                                         rhs=R4[:, ci * 484:(ci + 1) * 484],
                                         start=True, stop=True)
                        nc.scalar.copy(
                            out=Vsb[:, jp * PLANE + ci * 484: jp * PLANE + (ci + 1) * 484],
                            in_=vp)
                    # zero the pad-pixel positions (junk from window overhang)
                    # so BN2 stats and output pads are clean
                    bv = Vsb[:, jp * PLANE:(jp + 1) * PLANE].rearrange(
                        "p (y x) -> p y x", x=66)
                    nc.vector.memset(bv[:, 0, :], 0.0)
                    nc.vector.memset(bv[:, 65, :], 0.0)
                    nc.vector.memset(bv[:, 1:65, 0:1], 0.0)
                    nc.vector.memset(bv[:, 1:65, 65:66], 0.0)
                    for ci in range(9):
                        nc.vector.bn_stats(
                            out=VBN[:, (jp * 9 + ci) * 6:(jp * 9 + ci + 1) * 6],
                            in_=Vsb[:, jp * PLANE + ci * 484: jp * PLANE + (ci + 1) * 484])

                # per-row (mean, var) over all 8*4356 elements -> sums
                nc.vector.bn_aggr(out=VAG[:, :], in_=VBN[:, :])
                nV = float(8 * PLANE)
                vtmp = shared.tile([128, 1], dt.float32)
                nc.vector.tensor_scalar_mul(S2[:, 0:1], VAG[:, 0:1], nV)
                nc.vector.tensor_mul(vtmp, VAG[:, 0:1], VAG[:, 0:1])
                nc.vector.tensor_add(vtmp, VAG[:, 1:2], vtmp)
                nc.vector.tensor_scalar_mul(S2[:, 1:2], vtmp, nV)
                nc.sync.dma_start(out=b2i[:, :], in_=S2)
                if NOAR:
                    nc.sync.dma_start(out=b2o[:, :], in_=b2i[:, :])
                else:
                    nc.gpsimd.collective_compute(
                        "AllReduce", mybir.AluOpType.add,
                        replica_groups=[list(range(NCORES))],
                        ins=[b2i.opt()], outs=[b2o.opt()])
                nc.sync.dma_start(out=ar2, in_=b2o[:, :])
                if DEBUG:
                    nc.sync.dma_start(out=dbg_s2[:, :], in_=ar2)

                S32p = pst2.tile([32, 2], dt.float32)
                nc.tensor.matmul(out=S32p, lhsT=sel1, rhs=ar2, start=True, stop=True)
                ss = shared.tile([32, 2], dt.float32)
                nc.vector.tensor_copy(out=ss, in_=S32p)
                mean2 = shared.tile([32, 1], dt.float32)
                q2 = shared.tile([32, 1], dt.float32)
                v2 = shared.tile([32, 1], dt.float32)
                rc2 = shared.tile([32, 1], dt.float32)
                sc2 = shared.tile([32, 1], dt.float32)
                sh2 = shared.tile([32, 1], dt.float32)
                nc.vector.tensor_scalar_mul(mean2, ss[:, 0:1], 1.0 / NTOT)
                nc.vector.tensor_scalar_mul(q2, ss[:, 1:2], 1.0 / NTOT)
                nc.vector.tensor_mul(v2, mean2, mean2)
                nc.vector.tensor_sub(v2, q2, v2)
                nc.scalar.activation(out=v2, in_=v2, func=AF.Sqrt, bias=eps32, scale=1.0)
                nc.vector.reciprocal(out=rc2, in_=v2)
                nc.vector.tensor_mul(sc2, rc2, g2b2[:, 0:1])
                nc.vector.tensor_mul(sh2, mean2, sc2)
                nc.vector.tensor_sub(sh2, g2b2[:, 1:2], sh2)
                scsh = shared.tile([32, 2], dt.float32)
                nc.vector.tensor_copy(out=scsh[:, 0:1], in_=sc2)
                nc.vector.tensor_copy(out=scsh[:, 1:2], in_=sh2)
                BCp = pst2.tile([128, 2], dt.float32)
                nc.tensor.matmul(out=BCp, lhsT=sel2, rhs=scsh, start=True, stop=True)
                nc.vector.tensor_copy(out=bcB, in_=BCp)
                if DEBUG:
                    nc.sync.dma_start(out=dbg_bc[:, :], in_=bcB)

                for jp in range(8):
                    outSB = op.tile([128, PLANE], dt.float16, tag="o")
                    nc.vector.tensor_scalar(
                        outSB[:, :], Vsb[:, jp * PLANE:(jp + 1) * PLANE],
                        bcB[:, 0:1], bcB[:, 1:2], op0=ALU.mult, op1=ALU.add)
                    nc.sync.dma_start(out=outd[:, jp * PLANE:(jp + 1) * PLANE],
                                      in_=outSB)

    from waitfix_inline import fix_multiwaits
    fix_multiwaits(nc)
    # populate .instr bytes for extended-ISA instructions
    # (InstTensorTensorReduce) — without this the NEFF compiler fails with
    # "ISA wrong length"
    mybir.codegen_inst_isa_subclasses(nc)
    return nc


def _prep_core_inputs(x, a, h, w1, w2, bn2_g, bn2_b):
    # padded volume, z range [-2, 66)
    xp66 = np.zeros((B, C, 68, 66, 66), dtype=f16)
    xp66[:, :, 2:66, 1:65, 1:65] = x.astype(f16)

    W4 = np.zeros((128, 108), dtype=f16)
    w1h = w1.astype(f16)
    for q in range(4):
        W4[32 * q:32 * (q + 1), 27 * q:27 * (q + 1)] = w1h
    OBm = np.tile(np.eye(12, dtype=f16), (9, 1))            # row 12*e9+g
    W2B = np.zeros((108, 128), dtype=f16)
    w2h = w2.astype(f16)                                     # [27, 32]
    for g in range(4):
        W2B[27 * g:27 * (g + 1), 32 * g:32 * (g + 1)] = w2h
    sel1 = np.zeros((128, 32), dtype=np.float32)
    sel1[np.arange(128), np.arange(128) % 32] = 1.0
    sel2 = np.ascontiguousarray(sel1.T)
    g2b2 = np.ascontiguousarray(
        np.stack([bn2_g.astype(np.float32), bn2_b.astype(np.float32)], axis=1))
    mr = np.zeros((36, 1), np.float32)
    mr[2:34] = 1.0

    in_maps = []
    for k in range(NCORES):
        b, hh = k // 2, k % 2
        sl = xp66[b, :, 32 * hh:32 * hh + 36]                # [32, 36, 4356]
        sl = sl.reshape(C, 36, PLANE)
        # xin rows 32q+c, group gi blocks
        arr = sl.reshape(C, 9, 4, PLANE).transpose(1, 2, 0, 3)  # [9, 4, 32, px]
        arr = np.ascontiguousarray(arr.reshape(9, 128, PLANE).transpose(1, 0, 2)
                                   ).reshape(128, 9 * PLANE)
        zmv = np.ones((36, 1), np.float32)
        zmv[0] = zmv[35] = 0.0
        if hh == 0:
            zmv[1] = 0.0
        if hh == 1:
            zmv[34] = 0.0
        in_maps.append({"xin": arr, "w4in": W4, "obin": OBm, "w2in": W2B,
                        "sel1in": sel1, "sel2in": sel2, "zmin": zmv,
                        "mrin": mr, "g2b2in": g2b2})
    return in_maps


def _run_device(x, conv1_w, conv1_b, conv2_w, mu, sigma, bn1_g, bn1_b,
                bn2_g, bn2_b):
    from concourse.bass_utils import run_bass_kernel_spmd
    import os

    a, h, w1, w2 = _host_folds(conv1_w, conv1_b, conv2_w, mu, sigma)
    g1, b1 = float(bn1_g[0]), float(bn1_b[0])
    nc = _build(a, h, g1, b1)
    in_maps = _prep_core_inputs(x, a, h, w1, w2, bn2_g, bn2_b)

    trace = bool(os.environ.get("BASS_TRACE"))
    res = run_bass_kernel_spmd(nc, in_maps, core_ids=list(range(NCORES)),
                               trace=trace)
    global LAST_RES
    LAST_RES = res

    out = np.empty((B, C, D, H, W), np.float32)
    for k in range(NCORES):
        b, hh = k // 2, k % 2
        O = res.results[k]["out"].astype(np.float32).reshape(4, 32, 8, 66, 66)
        for jp in range(8):
            for g in range(4):
                out[b, :, 32 * hh + 4 * jp + g] = O[g, :, jp, 1:65, 1:65]
    return out


def _numpy_fallback(x, conv1_w, conv1_b, conv2_w, conv2_b, mu, sigma,
                    bn1_g, bn1_b, bn2_g, bn2_b):
    def conv3d(xx, w):
        b_, ci, d_, h_, wd = xx.shape
        o = w.shape[0]
        xp = np.zeros((b_, ci, d_ + 2, h_ + 2, wd + 2), np.float32)
        xp[:, :, 1:-1, 1:-1, 1:-1] = xx
        out = np.zeros((b_, o, d_, h_, wd), np.float32)
        for dz in range(3):
            for dy in range(3):
                for dx in range(3):
                    out += np.einsum(
                        "oc,bczyx->bozyx", w[:, :, dz, dy, dx],
                        xp[:, :, dz:dz + d_, dy:dy + h_, dx:dx + wd],
                        optimize=True)
        return out

    x = x.astype(np.float32)
    x1 = conv3d(x, conv1_w.astype(np.float32)) \
        + conv1_b.astype(np.float32)[None, :, None, None, None]
    m = mu[0].astype(np.float64)
    s = sigma[0].astype(np.float64)
    aa = np.sum(1.0 / s**2)
    bb = -2.0 * np.sum(m / s**2)
    cc = np.sum(m**2 / s**2)
    x1d = x1.astype(np.float64)
    fuzz = np.exp(-(aa * x1d * x1d + bb * x1d + cc)).astype(np.float32)
    mean1 = fuzz.mean(dtype=np.float64)
    var1 = np.mean((fuzz.astype(np.float64) - mean1) ** 2)
    y = ((fuzz - mean1) / np.sqrt(var1 + EPS)).astype(np.float32)
    y = bn1_g.astype(np.float32)[0] * y + bn1_b.astype(np.float32)[0]
    v = conv3d(y, conv2_w.astype(np.float32)) \
        + conv2_b.astype(np.float32)[None, :, None, None, None]
    vd = v.astype(np.float64)
    mean2 = vd.mean(axis=(0, 2, 3, 4))
    var2 = ((vd - mean2[None, :, None, None, None]) ** 2).mean(axis=(0, 2, 3, 4))
    sc = bn2_g.astype(np.float64) / np.sqrt(var2 + EPS)
    sh = bn2_b.astype(np.float64) - mean2 * sc
    return (vd * sc[None, :, None, None, None]
            + sh[None, :, None, None, None]).astype(np.float32)


def kernel(x, conv1_w, conv1_b, conv2_w, conv2_b, mu, sigma,
           bn1_g, bn1_b, bn2_g, bn2_b):
    x = np.asarray(x, np.float32)
    args = dict(
        x=x,
        conv1_w=np.asarray(conv1_w, np.float32),
        conv1_b=np.asarray(conv1_b, np.float32),
        conv2_w=np.asarray(conv2_w, np.float32),
        mu=np.asarray(mu, np.float32), sigma=np.asarray(sigma, np.float32),
        bn1_g=np.asarray(bn1_g, np.float32), bn1_b=np.asarray(bn1_b, np.float32),
        bn2_g=np.asarray(bn2_g, np.float32), bn2_b=np.asarray(bn2_b, np.float32),
    )
    try:
        out = _run_device(**args)
        global DEVICE_OK
        DEVICE_OK = True
        return out
    except Exception:
        import traceback
        traceback.print_exc()
        return _numpy_fallback(
            x, args["conv1_w"], args["conv1_b"], args["conv2_w"],
            np.asarray(conv2_b, np.float32), args["mu"], args["sigma"],
            args["bn1_g"], args["bn1_b"], args["bn2_g"], args["bn2_b"])


# ---- inline waitfix module (kernel must be self-contained) ----
import sys as _sys
import types as _types

_wf = _types.ModuleType("waitfix_inline")
_wf_code = '''
from concourse import mybir
_ctr = [0]
def fix_multiwaits(nc, keep_embedded=1):
    n_split = 0
    for f in nc.m.functions:
        for bb in f.blocks:
            out = []
            changed = False
            for inst in bb.instructions:
                si = inst.sync_info
                waits = list(si.on_wait) if si is not None else []
                if len(waits) > keep_embedded:
                    extra, keep = waits[:-keep_embedded], waits[-keep_embedded:]
                    for w in extra:
                        _ctr[0] += 1
                        ev = mybir.InstEventSemaphore(
                            name="waitsplit_%d" % _ctr[0],
                            ins=[], outs=[], engine=inst.engine,
                            sync_info=mybir.SyncInfo(on_wait=[w], on_update=[]),
                        )
                        out.append(ev)
                    inst.sync_info = mybir.SyncInfo(
                        on_wait=keep, on_update=list(si.on_update))
                    n_split += 1
                    changed = True
                out.append(inst)
            if changed:
                bb.instructions = out
    return n_split
'''
exec(_wf_code, _wf.__dict__)
_sys.modules["waitfix_inline"] = _wf
